# revision 4
# baseline (speedup 1.0000x reference)
"""Trainium2 Bass kernel for nn_MultiHeadAttention (B=2,S=128,H=16,W=16,E=256, 8 heads).

v2: natural-layout I/O (zero host-side tensor work), on-chip x transpose,
late softmax normalization folded into the AV-psum evacuation, natural-layout
out_proj. Sharding: H axis split 8 ways (2 h-slices per core = 64 (b,h,w)
attention problems per core, pure SPMD, no collectives).

Per-core pipeline (per group of 4 slices):
  - DMA x natural [s,e] fp32 (4KB rows), Pool cast->bf16, xbar transpose to xT
  - in_proj Q/K/V from xT (Q pre-scaled); Q^T,K^T evacuated to SBUF bf16
  - per slice: additive-mask matmul + 8 per-head K=32 QK^T matmuls, exp on ACT,
    row-sums (DVE+Pool), reciprocal, PE-transpose+broadcast of 1/sums,
    xbar transpose of P, per-head P^T@V, evac with fused 1/sum multiply,
  - out_proj with stationary=O^T chunks producing natural [s,e] rows,
    Pool evac, DMA store to natural y.
"""

import sys

import numpy as np

sys.path.insert(0, "/opt/trn_rl_repo")

from contextlib import ExitStack

import concourse.bass as bass
import concourse.mybir as mybir
import concourse.tile as tile
from concourse import bacc

P = 128
NCORES = 8
NSLICE = 64  # slices per core
GSL = 4  # slices per group
NG = NSLICE // GSL  # groups per core
NH = 8
HD = 32
E = 256
S = 128
B = 2
H = 16
W = 16
HLOC = H // NCORES  # 2

F32 = mybir.dt.float32
F32R = mybir.dt.float32r
BF16 = mybir.dt.bfloat16
AX = mybir.AxisListType
ALU = mybir.AluOpType
AF = mybir.ActivationFunctionType


def build_program(ng=NG, repeats=1):
    nc = bacc.Bacc("TRN2", target_bir_lowering=False, debug=False, num_devices=NCORES)

    x_d = nc.dram_tensor("x", [B, S, HLOC, W, E], F32, kind="ExternalInput").ap()
    wq_d = nc.dram_tensor("wq", [2, P, 256], BF16, kind="ExternalInput").ap()
    wk_d = nc.dram_tensor("wk", [2, P, 256], BF16, kind="ExternalInput").ap()
    wv_d = nc.dram_tensor("wv", [2, P, 256], BF16, kind="ExternalInput").ap()
    wot_d = nc.dram_tensor("wot", [2, P, 256], F32R, kind="ExternalInput").ap()
    am_d = nc.dram_tensor("amask", [P, S], BF16, kind="ExternalInput").ap()
    ni_d = nc.dram_tensor("negi", [P, 4 * S], BF16, kind="ExternalInput").ap()
    rm_d = nc.dram_tensor("rmask2", [P, 2], F32, kind="ExternalInput").ap()
    sb_d = nc.dram_tensor("selbd", [16, 4, P], BF16, kind="ExternalInput").ap()
    id_d = nc.dram_tensor("ident", [P, P], BF16, kind="ExternalInput").ap()
    idr_d = nc.dram_tensor("identr", [P, P], F32, kind="ExternalInput").ap()
    y_d = nc.dram_tensor("y", [B, S, HLOC, W, E], F32, kind="ExternalOutput").ap()

    with tile.TileContext(nc) as tc, ExitStack() as ctx:
        const = ctx.enter_context(tc.tile_pool(name="const", bufs=1))
        wq = const.tile([P, 2, 256], BF16, tag="wq")
        wk = const.tile([P, 2, 256], BF16, tag="wk")
        wv = const.tile([P, 2, 256], BF16, tag="wv")
        wot = const.tile([P, 2, 256], F32R, tag="wot")
        amask = const.tile([P, S], BF16, tag="amask")
        negi = const.tile([P, 4 * S], BF16, tag="negi")
        rmask2 = const.tile([P, 2], F32, tag="rmask2")
        selbd = const.tile([16, 4, P], BF16, tag="selbd")
        ident = const.tile([P, P], BF16, tag="ident")
        identr = const.tile([P, P], F32, tag="identr")
        nc.sync.dma_start(wq[:], wq_d.rearrange("c p f -> p c f"))
        nc.sync.dma_start(wk[:], wk_d.rearrange("c p f -> p c f"))
        nc.sync.dma_start(wv[:], wv_d.rearrange("c p f -> p c f"))
        nc.sync.dma_start(wot[:], wot_d.rearrange("c p f -> p c f"))
        nc.sync.dma_start(amask[:], am_d)
        nc.sync.dma_start(negi[:], ni_d)
        nc.sync.dma_start(rmask2[:], rm_d)
        nc.sync.dma_start(selbd[:], sb_d)
        nc.sync.dma_start(ident[:], id_d)
        nc.sync.dma_start(identr[:], idr_d)

        xp = ctx.enter_context(tc.tile_pool(name="xp", bufs=3))
        xbp = ctx.enter_context(tc.tile_pool(name="xbp", bufs=3))
        xtp = ctx.enter_context(tc.tile_pool(name="xtp", bufs=3))
        qkp = ctx.enter_context(tc.tile_pool(name="qkp", bufs=3))
        vp = ctx.enter_context(tc.tile_pool(name="vp", bufs=3))
        pp = ctx.enter_context(tc.tile_pool(name="pp", bufs=4))
        ptp = ctx.enter_context(tc.tile_pool(name="ptp", bufs=5))
        smp = ctx.enter_context(tc.tile_pool(name="smp", bufs=6))
        rtp = ctx.enter_context(tc.tile_pool(name="rtp", bufs=2))
        bcp = ctx.enter_context(tc.tile_pool(name="bcp", bufs=3))
        otp = ctx.enter_context(tc.tile_pool(name="otp", bufs=5))
        ysp = ctx.enter_context(tc.tile_pool(name="ysp", bufs=4))

        # PSUM: per-stream pools so phases don't serialize on slot rotation
        ps_sc = ctx.enter_context(tc.tile_pool(name="ps_sc", bufs=3, space="PSUM"))
        ps_io = ctx.enter_context(tc.tile_pool(name="ps_io", bufs=3, space="PSUM"))
        ps_av = ctx.enter_context(tc.tile_pool(name="ps_av", bufs=2, space="PSUM"))

        def g_bhw(g):
            return g >> 3, (g >> 2) & 1, (g & 3) * GSL

        def emit_x_load(g):
            b, h, w0 = g_bhw(g)
            xg = xp.tile([P, GSL, 256], F32, tag="xg")
            nc.sync.dma_start(xg[:], x_d[b][:, h, w0 : w0 + GSL, :])
            return xg

        def emit_xt(xg):
            xb = xbp.tile([P, GSL, 256], BF16, tag="xb")
            nc.gpsimd.tensor_copy(
                xb[:].rearrange("p a b -> p (a b)"),
                xg[:].rearrange("p a b -> p (a b)"),
            )
            # xT layout [e128, sl, ec, s] so the 8 xbar blocks land in order
            xt = xtp.tile([P, GSL, 2, S], BF16, tag="xt")
            nc.sync.dma_start_transpose(
                xt[:].rearrange("p a b c -> p (a b) c"),
                xb[:].rearrange("p a b -> p (a b)"),
            )
            return xt

        def emit_in_proj(xt):
            # Q^T / zeroed K^T stored as 64-partition head-pair blocks so all
            # score matmuls use base partition 0 (mixed bases cannot share a
            # PSUM accumulation tile).
            qt = qkp.tile([64, 2, 2, GSL, S], BF16, tag="qt")
            ktd = qkp.tile([P, 2, GSL, S], BF16, tag="ktd")
            kbd = [
                qkp.tile([64, 2, 2, GSL, S], BF16, tag="kbd0", name="kbd0"),
                qkp.tile([64, 2, 2, GSL, S], BF16, tag="kbd1", name="kbd1"),
            ]
            for which, wmat in ((0, wq), (1, wk)):
                for ft in range(2):
                    ps = ps_io.tile([P, GSL * S], F32, tag="io")
                    for ec in range(2):
                        nc.tensor.matmul(
                            ps[:],
                            lhsT=wmat[:, ec, ft * P : (ft + 1) * P],
                            rhs=xt[:, :, ec, :],
                            start=(ec == 0),
                            stop=(ec == 1),
                        )
                    if which == 0:
                        for b in range(2):
                            nc.vector.tensor_copy(
                                qt[:, b, ft].rearrange("p a b -> p (a b)"),
                                ps[64 * b : 64 * b + 64, :],
                            )
                    else:
                        nc.scalar.copy(
                            ktd[:, ft].rearrange("p a b -> p (a b)"), ps[:]
                        )
                        for b in range(2):
                            for j2 in range(2):
                                dst = kbd[b][:, ft, j2].rearrange(
                                    "p a b -> p (a b)"
                                )
                                nc.gpsimd.tensor_scalar(
                                    dst,
                                    ktd[64 * b : 64 * b + 64, ft].rearrange(
                                        "p a b -> p (a b)"
                                    ),
                                    rmask2[0:64, j2 : j2 + 1],
                                    None,
                                    ALU.mult,
                                )
            v = vp.tile([P, GSL, 256], BF16, tag="v")
            for slp in range(GSL // 2):
                psv = ps_io.tile([P, 2, 256], F32, tag="io")
                for half in range(2):
                    sl = slp * 2 + half
                    for ec in range(2):
                        nc.tensor.matmul(
                            psv[:, half],
                            lhsT=xt[:, sl, ec, :],
                            rhs=wv[:, ec, :],
                            start=(ec == 0),
                            stop=(ec == 1),
                        )
                nc.scalar.copy(v[:, slp * 2 : slp * 2 + 2, :], psv[:])
            return qt, kbd, v

        def emit_scores_pair(qt, kbd, sp):
            """Slice pair sp: mask + head-pair K=64 QK^T (PE), exp (ACT),
            row sums (DVE), paired all-head transpose (DMA xbar), and the
            per-pair 1/sum chain (DVE rcp, PE transpose, Pool evac)."""
            pe2 = pp.tile([P, 2, NH, S], BF16, tag="pe")
            for h2 in range(2):
                sl = 2 * sp + h2
                for hg in range(2):
                    ssc = ps_sc.tile([P, 4, S], F32, tag="sc")
                    nc.tensor.matmul(
                        ssc[:].rearrange("p a b -> p (a b)"),
                        lhsT=amask[:],
                        rhs=negi[:],
                        start=True,
                        stop=False,
                        skip_group_check=True,
                    )
                    for b in range(2):
                        nc.tensor.matmul(
                            ssc[:, 2 * b : 2 * b + 2, :],
                            lhsT=qt[:, b, hg, sl, :],
                            rhs=kbd[b][:, hg, :, sl, :],
                            start=False,
                            stop=True,
                            skip_group_check=True,
                        )
                    nc.scalar.activation(
                        pe2[:, h2, hg * 4 : hg * 4 + 4, :], ssc[:], AF.Exp
                    )
            pt2 = ptp.tile([P, 2, NH, S], BF16, tag="pt")
            nc.sync.dma_start_transpose(
                pt2[:].rearrange("p a b c -> p (a b) c"),
                pe2[:].rearrange("p a b c -> p (a b c)"),
            )
            return pt2, pe2

        def emit_sums(pe2):
            """DVE row-sums + reciprocal (emitted late so the DVE queue sees
            the current group's tt evacuations first)."""
            sums = smp.tile([P, 2, NH], F32, tag="sums")
            nc.vector.tensor_reduce(sums[:], pe2[:], axis=AX.X, op=ALU.add)
            rcp = smp.tile([P, 2 * NH], F32, tag="rcp")
            nc.vector.reciprocal(rcp[:], sums[:].rearrange("p a b -> p (a b)"))
            return rcp

        def emit_av_pair(v, pt2, rcp, sp):
            """1/sum transpose+broadcast (PE), AV matmuls, fused normalize
            evacuation (DVE tensor_tensor)."""
            rtps = ps_av.tile([2 * NH, P], F32, tag="av")
            nc.tensor.transpose(rtps[:], rcp[:], identr[:])
            rcpt = rtp.tile([2 * NH, P], BF16, tag="rt")
            nc.scalar.copy(rcpt[:], rtps[:])
            bc2 = ps_av.tile([P, 2, 2, S], F32, tag="av")
            for h2 in range(2):
                sl = 2 * sp + h2
                for hg in range(2):
                    nc.tensor.matmul(
                        bc2[:, h2, hg, :],
                        lhsT=selbd[:, h2 * 2 + hg, :],
                        rhs=rcpt[:],
                    )
            po2 = ps_av.tile([P, 2, 2, S], F32, tag="av")
            for h2 in range(2):
                sl = 2 * sp + h2
                for hg in range(2):
                    for j in range(4):
                        i = hg * 4 + j
                        o32 = 32 * j
                        nc.tensor.matmul(
                            po2[o32 : o32 + 32, h2, hg, :],
                            lhsT=v[:, sl, i * 32 : (i + 1) * 32],
                            rhs=pt2[:, h2, i, :],
                            tile_position=(0, o32),
                        )
            bcsb = bcp.tile([P, 2, 2, S], BF16, tag="bcsb")
            nc.scalar.copy(
                bcsb[:].rearrange("p a b c -> p (a b c)"),
                bc2[:].rearrange("p a b c -> p (a b c)"),
            )
            ot2 = otp.tile([P, 2, 2, S], F32R, tag="ot")
            nc.vector.tensor_tensor(
                ot2[:].rearrange("p a b c -> p (a b c)"),
                po2[:].rearrange("p a b c -> p (a b c)"),
                bcsb[:].rearrange("p a b c -> p (a b c)"),
                ALU.mult,
            )
            return ot2

        def emit_out_proj_pair(ot2, g, sp):
            b, h, w0 = g_bhw(g)
            py2 = ps_av.tile([P, 2, 256], F32, tag="av")
            for h2 in range(2):
                for ec in range(2):
                    nc.tensor.matmul(
                        py2[:, h2, :],
                        lhsT=ot2[:, h2, ec, :],
                        rhs=wot[:, ec, :],
                        start=(ec == 0),
                        stop=(ec == 1),
                    )
            ysb = ysp.tile([P, 2, 256], F32, tag="ysb")
            yeng = nc.scalar.copy if sp == 0 else nc.vector.tensor_copy
            yeng(
                ysb[:].rearrange("p a b -> p (a b)"),
                py2[:].rearrange("p a b -> p (a b)"),
            )
            w1 = w0 + 2 * sp
            nc.sync.dma_start(y_d[b][:, h, w1 : w1 + 2, :], ysb[:])

        def emit_group_front(g):
            """x load + in_proj for group g."""
            xg = emit_x_load(g)
            xt = emit_xt(xg)
            return emit_in_proj(xt)

        def emit_group_scores(qkv):
            qt, kbd, v = qkv
            out = []
            for sp in range(2):
                pt2, pe2 = emit_scores_pair(qt, kbd, sp)
                out.append((pt2, emit_sums(pe2)))
            return out

        # ---- two-group-deep software pipeline, finely interleaved so each
        # engine's FIFO queue alternates scores(g+1) / AV+out(g) work
        for _rep in range(repeats):
            qkv = {}
            sc = {}
            qkv[0] = emit_group_front(0)
            sc[0] = emit_group_scores(qkv[0])
            if ng > 1:
                qkv[1] = emit_group_front(1)
            for g in range(ng):
                v = qkv[g][2]
                if g + 2 < ng:
                    xg2 = emit_x_load(g + 2)
                if g + 1 < ng:
                    qt1, kbd1, _ = qkv[g + 1]
                    npts = []
                for sp in range(2):
                    if g + 1 < ng:
                        pt2n, pe2n = emit_scores_pair(qt1, kbd1, sp)
                    pt2, rcp = sc[g][sp]
                    ot2 = emit_av_pair(v, pt2, rcp, sp)
                    emit_out_proj_pair(ot2, g, sp)
                    if g + 1 < ng:
                        npts.append((pt2n, emit_sums(pe2n)))
                    if sp == 0 and g + 2 < ng:
                        xt2 = emit_xt(xg2)
                        qkv[g + 2] = emit_in_proj(xt2)
                if g + 1 < ng:
                    sc[g + 1] = npts
                del qkv[g], sc[g]

    nc.compile()
    return nc


def prep_weights(w_in, w_out):
    """Host-side prep of the small projection weights (~0.8MB of numpy)."""
    import ml_dtypes

    bf16 = ml_dtypes.bfloat16
    scale = 1.0 / np.sqrt(HD)
    idx_q = np.concatenate([np.arange(i * 96, i * 96 + 32) for i in range(NH)])

    def pack_w(Wm, dt):
        # lhsT layout [ec, ep, f]
        return np.ascontiguousarray(Wm.T.reshape(2, P, 256)).astype(dt)

    wq_h = pack_w(w_in[idx_q] * scale, bf16)
    wk_h = pack_w(w_in[idx_q + 32], bf16)
    wv_h = pack_w(w_in[idx_q + 64], bf16)
    wot_h = np.ascontiguousarray(w_out.T.reshape(2, P, 256)).astype(np.float32)
    return wq_h, wk_h, wv_h, wot_h


def make_consts():
    import ml_dtypes

    bf16 = ml_dtypes.bfloat16
    am_h = np.tril(np.ones((S, S), np.float32), -1).astype(bf16)
    ni_h = np.ascontiguousarray(
        (-1000.0 * np.eye(S, dtype=np.float32))[:, None, :]
        .repeat(4, 1)
        .reshape(S, 4 * S)
    ).astype(bf16)
    rm2_h = np.zeros((P, 2), np.float32)
    for f in range(P):
        rm2_h[f, (f // 32) % 2] = 1.0
    sel_h = np.zeros((16, 4, P), np.float32)
    for t in range(4):
        for m in range(P):
            sel_h[t * 4 + m // 32, t, m] = 1.0
    sel_h = sel_h.astype(bf16)
    id_h = np.eye(P, dtype=np.float32).astype(bf16)
    idr_h = np.eye(P, dtype=np.float32)
    return am_h, ni_h, rm2_h, sel_h, id_h, idr_h


_NC_CACHE = {}


def get_program(repeats=1):
    if repeats not in _NC_CACHE:
        _NC_CACHE[repeats] = build_program(repeats=repeats)
    return _NC_CACHE[repeats]


class _Executor:
    """Cached PJRT executor.

    jit(shard_map(bass_exec)) with natural-layout global args:
      x [2,128,16,16,256] f32 sharded on H, weights replicated, constant
      masks + zero output buffers as cached committed device arrays.
    """

    def __init__(self, nc):
        import jax
        from jax.sharding import Mesh, NamedSharding, PartitionSpec
        from jax.experimental.shard_map import shard_map
        from concourse.bass2jax import _bass_exec_p, install_neuronx_cc_hook

        install_neuronx_cc_hook()
        self.nc = nc
        pname = nc.partition_id_tensor.name if nc.partition_id_tensor else None
        in_names, out_names, out_avals = [], [], []
        for alloc in nc.m.functions[0].allocations:
            if not isinstance(alloc, mybir.MemoryLocationSet):
                continue
            name = alloc.memorylocations[0].name
            if alloc.kind == "ExternalInput":
                if name != pname:
                    in_names.append(name)
            elif alloc.kind == "ExternalOutput":
                out_names.append(name)
                shape = tuple(alloc.tensor_shape)
                dtype = mybir.dt.np(alloc.dtype)
                out_avals.append(jax.core.ShapedArray(shape, dtype))
        assert in_names == [
            "x", "wq", "wk", "wv", "wot", "amask", "negi", "rmask2", "selbd",
            "ident", "identr",
        ], in_names
        assert out_names == ["y"], out_names
        all_names = in_names + out_names + ([pname] if pname else [])

        from concourse.bass2jax import partition_id_tensor

        def _body(*args):
            operands = list(args)
            if pname is not None:
                operands.append(partition_id_tensor())
            return tuple(
                _bass_exec_p.bind(
                    *operands,
                    out_avals=tuple(out_avals),
                    in_names=tuple(all_names),
                    out_names=tuple(out_names),
                    lowering_input_output_aliases=(),
                    sim_require_finite=True,
                    sim_require_nnan=True,
                    nc=nc,
                )
            )

        devices = jax.devices()[:NCORES]
        mesh = Mesh(np.asarray(devices), ("core",))
        px = PartitionSpec(None, None, "core", None, None)
        pr = PartitionSpec()
        in_specs = (px,) + (pr,) * 10 + (px,)
        self._jit = jax.jit(
            shard_map(
                _body,
                mesh=mesh,
                in_specs=in_specs,
                out_specs=(px,),
                check_rep=False,
            ),
            keep_unused=True,
        )
        # Cached committed device arrays: constant masks (replicated) and the
        # zero output carrier (sharded). Not donated, so reusable every call.
        consts = make_consts()
        rep = NamedSharding(mesh, pr)
        shx = NamedSharding(mesh, px)
        self._consts = tuple(jax.device_put(c, rep) for c in consts)
        self._zero_y = jax.device_put(
            np.zeros((B, S, H, W, E), np.float32), shx
        )
        self._jax = jax

    def run(self, x, wq, wk, wv, wot):
        y = self._jit(x, wq, wk, wv, wot, *self._consts, self._zero_y)[0]
        return np.asarray(y)


_EXEC_CACHE = {}


def get_executor(repeats=1):
    if repeats not in _EXEC_CACHE:
        _EXEC_CACHE[repeats] = _Executor(get_program(repeats))
    return _EXEC_CACHE[repeats]


def kernel(hidden_state, w_in, w_out, repeats=1):
    hidden_state = np.asarray(hidden_state, dtype=np.float32)
    w_in = np.asarray(w_in, dtype=np.float32)
    w_out = np.asarray(w_out, dtype=np.float32)
    ex = get_executor(repeats)
    wq_h, wk_h, wv_h, wot_h = prep_weights(w_in, w_out)
    return ex.run(hidden_state, wq_h, wk_h, wv_h, wot_h)
